# revision 14
# baseline (speedup 1.0000x reference)
"""MoE MLP (pre-LN + top-4-of-8 router + per-expert FFN) on 8 Trainium2 NeuronCores.

Sharding: data-parallel over tokens (4096 tokens/core), weights replicated.
The host routing plan (needed anyway to build the sparse schedule) assigns
tokens to cores so per-(core,expert) routed counts are nearly equal (minimal
tile padding), orders tokens within each core by first routed expert (so
expert 0's FFN starts when LayerNorm is only ~half done), and supplies the
exact fp32 top-4 softmax gates in slot order.

Per-core pipeline:
  phase 1 (per 128-token tile): LayerNorm (bn_stats/bn_aggr + scalar-engine
    fused normalize) -> spill xn rows (bf16, 2KB) to DRAM. No PE work.
  phase 2 (per expert): w1 + w2 (bf16, chunked DMAs) prefetched one block
    early on the scalar DMA ring; per <=512-token block: indirect-gather xn
    rows (prefetched two blocks ahead), PE-transpose, hT = gelu(w1.T@xnT+b1),
    c_tile = (hT.T @ w2) * gate, stored bf16 to a per-expert contributions
    DRAM tensor at static offsets (no RMW scatter).
  host: y[token] = sum of its 4 expert contributions (the expert-parallel
    "all-reduce the weighted accumulation" step, done during unshard).
"""

import numpy as np
import ml_dtypes

import concourse.bass as bass
import concourse.mybir as mybir
import concourse.tile as tile
from concourse import bacc
from concourse.bass_utils import run_bass_kernel_spmd
from concourse.masks import make_identity

# Problem shape (fixed by the task).
T, D, H, OUT = 32768, 1024, 2048, 1024
E, K = 8, 4
EPS = 1e-5

NCORES = 8
P = 128
TLOC = T // NCORES          # tokens per core
NTILE = TLOC // P           # 128-token tiles per core (32)
DC = D // P                 # 8 contraction chunks for D
HC = H // P                 # 16 chunks for H
OC = OUT // 512             # 2 output column blocks of 512

BF = mybir.dt.bfloat16
F32 = mybir.dt.float32

_PROGRAM_CACHE = {}

# test-harness hooks (ignored by graders that just call kernel()):
TRACE = False           # set True to request an NTFF trace / HW exec time
LAST_RESULTS = None     # BassKernelResults of the most recent run


BLK = 512
JT = BLK // P            # 128-token sub-tiles per block
PSH_BUFS = 4
OOB = TLOC               # pad index: one past the last valid row (skipped via bounds_check)


def _expert_blocks(ntiles: int, last_bsz: int, tail_small: bool):
    """Split an expert's slot tiles into (offset, jn, bsz) chunks of <=4 tiles,
    distributed evenly so no chunk is tiny. The final chunk's bsz is the exact
    max token count so padded slots aren't streamed through the matmuls. With
    tail_small (last expert), the final chunk is a single tile so the kernel's
    drain tail is short."""
    tail = []
    if tail_small and ntiles > 1:
        tail = [(ntiles - 1, 1, last_bsz - (ntiles - 1) * P)]
        ntiles -= 1
        last_bsz = ntiles * P
    nb = (ntiles + 3) // 4
    base, rem = divmod(ntiles, nb)
    blocks, s0 = [], 0
    for b in range(nb):
        jn = base + (1 if b < rem else 0)
        bsz = jn * P
        if b == nb - 1:
            bsz = last_bsz - s0 * P
            assert (jn - 1) * P < bsz <= jn * P
        blocks.append((s0, jn, bsz))
        s0 += jn
    return blocks + tail


def _make_blocks(tiles_per_expert, last_bsz):
    blocks = []
    for e in range(E):
        for s0, jn, bsz in _expert_blocks(tiles_per_expert[e], last_bsz[e],
                                          e == E - 1):
            blocks.append((e, s0, jn, bsz))
    return blocks


def build_sparse_program(apply_gamma: bool, apply_beta: bool,
                         tiles_per_expert: tuple, last_bsz: tuple,
                         block_lims: tuple):
    """Top-4 sparse FFN. Host supplies per-expert token index lists (padded
    with OOB), slot-ordered fp32 gates, and per expert: the tile count, exact
    max token count, and how many 128-row xn tiles its gathers may touch
    (tokens are ordered by first routed expert). Expert contributions are
    stored (bf16) to per-expert DRAM tensors; host does the final sum."""
    NSLOT = max(tiles_per_expert)
    nc = bacc.Bacc(None, target_bir_lowering=False, debug=False, num_devices=NCORES)

    x = nc.declare_dram_parameter("x", [TLOC, D], BF, isOutput=False)
    w1t = nc.declare_dram_parameter("w1t", [E, P, DC, H], BF, isOutput=False)
    w2t = nc.declare_dram_parameter("w2t", [E, P, HC, OUT], BF, isOutput=False)
    b1t = nc.declare_dram_parameter("b1t", [E, P, HC], F32, isOutput=False)
    gamma = nc.declare_dram_parameter("gamma", [D], F32, isOutput=False)
    beta = nc.declare_dram_parameter("beta", [D], F32, isOutput=False)
    idxt = nc.declare_dram_parameter("idxt", [P, E, NSLOT], mybir.dt.int32,
                                     isOutput=False)
    gslot = nc.declare_dram_parameter("gslot", [P, E, NSLOT], F32,
                                      isOutput=False)

    contribs = [
        nc.declare_dram_parameter(f"contrib{e}", [tiles_per_expert[e] * P, OUT],
                                  BF, isOutput=True)
        for e in range(E)
    ]

    xn_dram = nc.dram_tensor("xn_dram", [TLOC, D], BF)

    blocks = _make_blocks(tiles_per_expert, last_bsz)
    NB = len(blocks)
    assert len(block_lims) == NB

    with tile.TileContext(nc) as tc:
        with (
            tc.tile_pool(name="singles", bufs=1) as singles,
            tc.tile_pool(name="p1", bufs=3) as p1,
            tc.tile_pool(name="p1s", bufs=6) as p1s,
            tc.tile_pool(name="w1pool", bufs=2) as w1pool,
            tc.tile_pool(name="wpool", bufs=2) as wpool,
            tc.tile_pool(name="hpool", bufs=2) as hpool,
            tc.tile_pool(name="ypool", bufs=2) as ypool,
            tc.tile_pool(name="xgtpool", bufs=2) as xgtpool,
            tc.tile_pool(name="xgpool", bufs=6) as xgpool,
            tc.tile_pool(name="ps_tr", bufs=2, space="PSUM") as ps_tr,
            tc.tile_pool(name="ps_h", bufs=PSH_BUFS, space="PSUM") as ps_h,
            tc.tile_pool(name="ps_y", bufs=2, space="PSUM") as ps_y,
        ):
            ident_bf = singles.tile([P, P], BF)
            make_identity(nc, ident_bf)
            eps_t = singles.tile([P, 1], F32)
            nc.vector.memset(eps_t, EPS)
            idx_sb = singles.tile([P, E, NSLOT], mybir.dt.int32)
            nc.gpsimd.dma_start(out=idx_sb, in_=idxt[:, :, :])
            g_sb = singles.tile([P, E, NSLOT], F32)
            nc.gpsimd.dma_start(out=g_sb, in_=gslot[:, :, :])


            if apply_gamma:
                gam_sb = singles.tile([P, D], F32)
                nc.sync.dma_start(
                    out=gam_sb,
                    in_=bass.AP(tensor=gamma.tensor, offset=gamma.offset,
                                ap=[[0, P], *gamma.ap]))
            if apply_beta:
                bet_sb = singles.tile([P, D], F32)
                nc.sync.dma_start(
                    out=bet_sb,
                    in_=bass.AP(tensor=beta.tensor, offset=beta.offset,
                                ap=[[0, P], *beta.ap]))

            # block 0 of expert 0 covers rows 0..511 exactly (tokens are
            # first-expert ordered), so phase-1 tiles 0..3 PE-transpose their
            # xn straight into its xgT: no spill-wait, no gather round-trip.
            b0_direct = (blocks[0] == (0, 0, JT, BLK))
            xgT0 = None
            if b0_direct:
                xgT0 = xgtpool.tile([P, DC, BLK], BF, tag="xgT",
                                    name="xgT0")

            # ---------------- phase 1 tile: LN + spill ------------------------
            def phase1_tile(i):
                xt = p1.tile([P, D], BF, tag="xt")
                nc.sync.dma_start(out=xt, in_=x[i * P:(i + 1) * P, :])

                st = p1s.tile([P, 2, 6], F32, tag="st")
                nc.vector.bn_stats(out=st[:, 0, :], in_=xt[:, 0:512])
                nc.vector.bn_stats(out=st[:, 1, :], in_=xt[:, 512:1024])
                mv = p1s.tile([P, 2], F32, tag="mv")
                nc.vector.bn_aggr(out=mv, in_=st)
                rstd = p1s.tile([P, 1], F32, tag="rstd")
                nc.scalar.activation(out=rstd, in_=mv[:, 1:2],
                                     func=mybir.ActivationFunctionType.Sqrt,
                                     bias=eps_t)
                nc.vector.reciprocal(out=rstd, in_=rstd)

                xnb = p1.tile([P, D], BF, tag="xnb")
                if apply_gamma or apply_beta:
                    nc.vector.tensor_scalar(out=xnb, in0=xt,
                                            scalar1=mv[:, 0:1], scalar2=rstd,
                                            op0=mybir.AluOpType.subtract,
                                            op1=mybir.AluOpType.mult)
                    if apply_gamma:
                        nc.vector.tensor_mul(out=xnb, in0=xnb, in1=gam_sb)
                    if apply_beta:
                        nc.vector.tensor_add(out=xnb, in0=xnb, in1=bet_sb)
                else:
                    # fused LN on the scalar engine: xn = x*rstd + (-mu*rstd)
                    negmr = p1s.tile([P, 1], F32, tag="negmr")
                    nc.vector.tensor_scalar(out=negmr, in0=mv[:, 0:1],
                                            scalar1=rstd, scalar2=-1.0,
                                            op0=mybir.AluOpType.mult,
                                            op1=mybir.AluOpType.mult)
                    for hf in range(2):
                        nc.scalar.activation(
                            out=xnb[:, hf * 512:(hf + 1) * 512],
                            in_=xt[:, hf * 512:(hf + 1) * 512],
                            func=mybir.ActivationFunctionType.Identity,
                            scale=rstd, bias=negmr)
                nc.gpsimd.dma_start(out=xn_dram[i * P:(i + 1) * P, :], in_=xnb)
                if b0_direct and i < JT:
                    for dc in range(DC):
                        pstb = ps_tr.tile([P, P], BF, tag="pstb")
                        nc.tensor.transpose(pstb, xnb[:, dc * P:(dc + 1) * P],
                                            ident_bf)
                        nc.vector.tensor_copy(
                            out=xgT0[:, dc, i * P:(i + 1) * P], in_=pstb)

            emitted = [0]

            def ensure_tiles(n):
                while emitted[0] < min(n, NTILE):
                    phase1_tile(emitted[0])
                    emitted[0] += 1

            # ---------------- phase 2: per-expert sparse FFN -------------------
            # Gathers run two blocks ahead (xg ring throttles); stage A(b) =
            # transpose+w1+gelu; stage B(b) = w2+gate-scale+contrib store;
            # A(b+1) issued before B(b). Next expert's weights are DMA'd on the
            # scalar ring one block before they're needed (bufs sized so the
            # ring slot belongs to the expert before last -> no issue-order
            # hazard). Remaining LN tiles are drip-fed between blocks.

            expert_state = {}

            # weight loads are chunked into ~512KB pieces so their DMA
            # packets round-robin fairly with the small x/xn/gather DMAs
            # (one 4MB dma_start = 32KB descriptors that hog each SDMA
            # engine for ~10us at packet granularity)
            def load_w1(e):
                w1sb = w1pool.tile([P, DC, H], BF, tag="w1")
                for k in range(DC):
                    nc.scalar.dma_start(out=w1sb[:, k, :], in_=w1t[e, :, k, :])
                b1sb = p1.tile([P, HC], F32, tag="b1")
                nc.scalar.dma_start(out=b1sb, in_=b1t[e, :, :])
                expert_state[e] = {"w1sb": w1sb, "b1sb": b1sb}

            def load_w2(e):
                w2sb = wpool.tile([P, HC, OUT], BF, tag="w2")
                for k in range(DC):
                    nc.scalar.dma_start(out=w2sb[:, 2 * k:2 * k + 2, :],
                                        in_=w2t[e, :, 2 * k:2 * k + 2, :])
                expert_state[e]["w2sb"] = w2sb

            gq = {}
            gathered = [-1]

            def prefetch_gathers(upto):
                while gathered[0] < min(upto, NB - 1):
                    gathered[0] += 1
                    bi = gathered[0]
                    e, s0, jn, bsz = blocks[bi]
                    ensure_tiles(block_lims[bi])
                    if bi == 0 and b0_direct:
                        gq[bi] = None
                        continue
                    lim = block_lims[bi] * P
                    tiles = []
                    for j in range(jn):
                        idx_ap = idx_sb[:, e, s0 + j:s0 + j + 1]
                        xg = xgpool.tile([P, D], BF, tag="xg")
                        nc.gpsimd.indirect_dma_start(
                            out=xg[:, :], out_offset=None,
                            in_=xn_dram[0:lim, :],
                            in_offset=bass.IndirectOffsetOnAxis(ap=idx_ap,
                                                                axis=0),
                            bounds_check=lim - 1, oob_is_err=False)
                        tiles.append(xg)
                    gq[bi] = tiles

            def stage_a(bi):
                e, s0, jn, bsz = blocks[bi]
                st = expert_state[e]
                xgs = gq.pop(bi)
                if bi == 0 and b0_direct:
                    xgT = xgT0
                else:
                    xgT = xgtpool.tile([P, DC, BLK], BF, tag="xgT")
                    for j in range(jn):
                        for dc in range(DC):
                            pstb = ps_tr.tile([P, P], BF, tag="pstb")
                            nc.tensor.transpose(
                                pstb, xgs[j][:, dc * P:(dc + 1) * P], ident_bf)
                            nc.vector.tensor_copy(
                                out=xgT[:, dc, j * P:(j + 1) * P], in_=pstb)

                hT = hpool.tile([P, HC, BLK], BF, tag="hT")
                for hc in range(HC):
                    psh = ps_h.tile([P, BLK], F32, tag="psh")
                    for dc in range(DC):
                        nc.tensor.matmul(
                            psh[:, :bsz],
                            lhsT=st["w1sb"][:, dc, hc * P:(hc + 1) * P],
                            rhs=xgT[:, dc, :bsz],
                            start=(dc == 0), stop=(dc == DC - 1))
                    nc.scalar.activation(out=hT[:, hc, :bsz], in_=psh[:, :bsz],
                                         func=mybir.ActivationFunctionType.Gelu,
                                         bias=st["b1sb"][:, hc:hc + 1])
                return (e, s0, jn, bsz, hT, st)

            def stage_b(pend):
                e, s0, jn, bsz, hT, st = pend
                for j in range(jn):
                    rows = min(P, bsz - j * P)
                    yt = ypool.tile([P, OUT], BF, tag="yt")
                    for oc in range(OC):
                        psy = ps_y.tile([P, 512], F32, tag="psy")
                        for hc in range(HC):
                            nc.tensor.matmul(
                                psy[:rows, :],
                                lhsT=hT[:, hc, j * P:j * P + rows],
                                rhs=st["w2sb"][:, hc, oc * 512:(oc + 1) * 512],
                                start=(hc == 0), stop=(hc == HC - 1))
                        nc.vector.tensor_scalar_mul(
                            out=yt[:rows, oc * 512:(oc + 1) * 512],
                            in0=psy[:rows, :],
                            scalar1=g_sb[:rows, e, s0 + j:s0 + j + 1])
                    r0 = (s0 + j) * P
                    nc.sync.dma_start(out=contribs[e][r0:r0 + rows, :],
                                      in_=yt[:rows, :])

            ensure_tiles(2)
            load_w1(0)
            pending = None
            for bi in range(NB):
                e = blocks[bi][0]
                if bi == 1:
                    load_w2(0)
                if bi + 1 < NB and blocks[bi + 1][0] != e:
                    load_w1(blocks[bi + 1][0])
                    load_w2(blocks[bi + 1][0])
                prefetch_gathers(bi + 2)
                nxt = stage_a(bi)
                if pending is not None:
                    stage_b(pending)
                pending = nxt
                ensure_tiles(emitted[0] + 3)
            stage_b(pending)

    nc.compile()
    return nc


def _assign_cores(sel):
    """Balanced token->core assignment: group tokens by expert signature and
    deal each group round-robin across cores, so per-(core,expert) routed
    counts are all ~N_e/8. Returns [T] core ids (exactly TLOC per core)."""
    sig = sel.astype(np.uint32).dot(1 << np.arange(E, dtype=np.uint32))
    order = np.argsort(sig, kind="stable")
    cores = np.empty(T, np.int64)
    cores[order] = np.arange(T, dtype=np.int64) % NCORES
    return cores


def _plan_routing(x, ln_gamma, ln_beta, router_w, router_b):
    """Host-side routing plan. Returns per-core token id lists (balanced
    assignment, ordered by first routed expert), per-(core,expert) index lists
    (positions within the core, padded with OOB), slot-ordered fp32 gates,
    per-expert tile counts / exact max counts / row limits, and the dense
    [T, E] gate matrix (for the b2 term)."""
    mu = x.mean(axis=1, keepdims=True)
    var = ((x - mu) ** 2).mean(axis=1, keepdims=True)
    xn = (x - mu) / np.sqrt(var + EPS) * ln_gamma + ln_beta
    logits = xn.astype(np.float32) @ router_w + router_b
    order = np.argsort(-logits, axis=1, kind="stable")[:, :K]     # [T, K]
    sel = np.zeros((x.shape[0], E), dtype=bool)
    np.put_along_axis(sel, order, True, axis=1)
    top = np.take_along_axis(logits, order, axis=1)
    gk = np.exp(top - top.max(axis=1, keepdims=True))
    gk = (gk / gk.sum(axis=1, keepdims=True)).astype(np.float32)
    gates = np.zeros((T, E), dtype=np.float32)
    np.put_along_axis(gates, order, gk, axis=1)

    cores = _assign_cores(sel)
    first_e = np.argmax(sel, axis=1)          # min routed expert per token

    tok_ids = []
    for c in range(NCORES):
        ids = np.nonzero(cores == c)[0]
        ids = ids[np.argsort(first_e[ids], kind="stable")]
        tok_ids.append(ids.astype(np.int64))

    counts = np.array([sel[tok_ids[c]].sum(axis=0) for c in range(NCORES)])
    tiles_per_expert = tuple(int(t) for t in
                             (counts.max(axis=0) + P - 1) // P)
    last_bsz = tuple(int(v) for v in counts.max(axis=0))
    nslot = max(tiles_per_expert)

    idxts, gslots, tok_lists, idx_raw = [], [], [], []
    for c in range(NCORES):
        sel_c = sel[tok_ids[c]]
        gates_c = gates[tok_ids[c]]
        idx = np.full((E, nslot * P), OOB, dtype=np.int32)
        gsl = np.zeros((E, nslot * P), dtype=np.float32)
        lists = []
        for e in range(E):
            toks = np.nonzero(sel_c[:, e])[0].astype(np.int32)
            idx[e, :toks.size] = toks
            gsl[e, :toks.size] = gates_c[toks, e]
            lists.append(toks)
        tok_lists.append(lists)
        # [E, nslot*P] -> [P, E, NSLOT] with slot s = (slot_tile, p)
        idxts.append(np.ascontiguousarray(
            idx.reshape(E, nslot, P).transpose(2, 0, 1)))
        gslots.append(np.ascontiguousarray(
            gsl.reshape(E, nslot, P).transpose(2, 0, 1)))
        idx_raw.append(idx)

    # per-block gather row limits: the highest row any core's indices in the
    # block touch (tile-rounded). Tokens are first-expert ordered, so early
    # experts' early blocks only need the first few LN tiles.
    block_lims = []
    for e, s0, jn, bsz in _make_blocks(tiles_per_expert, last_bsz):
        hi = 0
        for c in range(NCORES):
            seg = idx_raw[c][e, s0 * P:(s0 + jn) * P]
            seg = seg[seg < OOB]
            if seg.size:
                hi = max(hi, int(seg.max()) + 1)
        block_lims.append(max(1, -(-hi // P)))
    block_lims = tuple(block_lims)
    return (tok_ids, idxts, gslots, tok_lists, tiles_per_expert, last_bsz,
            block_lims, gates)


def _prep_weights(w1, w2, b1):
    w1t = np.ascontiguousarray(
        w1.reshape(E, DC, P, H).transpose(0, 2, 1, 3)).astype(ml_dtypes.bfloat16)
    w2t = np.ascontiguousarray(
        w2.reshape(E, HC, P, OUT).transpose(0, 2, 1, 3)).astype(ml_dtypes.bfloat16)
    b1t = np.ascontiguousarray(
        b1.reshape(E, HC, P).transpose(0, 2, 1)).astype(np.float32)
    return w1t, w2t, b1t


def kernel(x, ln_gamma, ln_beta, router_w, router_b, w1, b1, w2, b2):
    x = np.asarray(x, dtype=np.float32)
    ln_gamma = np.asarray(ln_gamma, dtype=np.float32)
    ln_beta = np.asarray(ln_beta, dtype=np.float32)
    router_w = np.asarray(router_w, dtype=np.float32)
    router_b = np.asarray(router_b, dtype=np.float32)
    w1 = np.asarray(w1, dtype=np.float32)
    b1 = np.asarray(b1, dtype=np.float32)
    w2 = np.asarray(w2, dtype=np.float32)
    b2 = np.asarray(b2, dtype=np.float32)

    apply_gamma = not np.all(ln_gamma == 1.0)
    apply_beta = not np.all(ln_beta == 0.0)

    (tok_ids, idxts, gslots, tok_lists, tiles_per_expert, last_bsz,
     block_lims, gates) = _plan_routing(x, ln_gamma, ln_beta, router_w,
                                        router_b)

    flags = (apply_gamma, apply_beta)
    key = ("v6", *flags, tiles_per_expert, last_bsz, block_lims)
    if key not in _PROGRAM_CACHE:
        _PROGRAM_CACHE[key] = build_sparse_program(
            *flags, tiles_per_expert, last_bsz, block_lims)
    nc = _PROGRAM_CACHE[key]

    w1t, w2t, b1t = _prep_weights(w1, w2, b1)

    in_maps = []
    for c in range(NCORES):
        m = {
            "x": np.ascontiguousarray(
                x[tok_ids[c]].astype(ml_dtypes.bfloat16)),
            "w1t": w1t, "w2t": w2t, "b1t": b1t,
            "gamma": ln_gamma, "beta": ln_beta,
            "idxt": idxts[c],
            "gslot": gslots[c],
        }
        in_maps.append(m)

    global LAST_RESULTS
    res = run_bass_kernel_spmd(nc, in_maps, list(range(NCORES)), trace=TRACE)
    LAST_RESULTS = res

    # unshard + the expert-parallel weighted-accumulation reduction
    y = np.zeros((T, OUT), dtype=np.float32)
    for c in range(NCORES):
        rc = res.results[c]
        ids = tok_ids[c]
        for e in range(E):
            toks = tok_lists[c][e]
            if toks.size == 0:
                continue
            contrib = np.asarray(rc[f"contrib{e}"][:toks.size], dtype=np.float32)
            y[ids[toks]] += contrib

    if not np.all(b2 == 0.0):
        y = y + gates @ b2
    return y.astype(np.float32)


# revision 15
# speedup vs baseline: 1.1865x; 1.1865x over previous
"""MoE MLP (pre-LN + top-4-of-8 router + per-expert FFN) on 8 Trainium2 NeuronCores.

Sharding: data-parallel over tokens (4096 tokens/core), weights replicated.
The host routing plan (needed anyway to build the sparse schedule) assigns
tokens to cores so per-(core,expert) routed counts are nearly equal (minimal
tile padding), orders tokens within each core by first routed expert (so
expert 0's FFN starts when LayerNorm is only ~half done), and supplies the
exact fp32 top-4 softmax gates in slot order.

Per-core pipeline:
  phase 1 (per 128-token tile): LayerNorm (bn_stats/bn_aggr + scalar-engine
    fused normalize) -> spill xn rows (bf16, 2KB) to DRAM. No PE work.
  phase 2 (per expert): w1 + w2 (bf16, chunked DMAs) prefetched one block
    early on the scalar DMA ring; per <=512-token block: indirect-gather xn
    rows (prefetched two blocks ahead), PE-transpose, hT = gelu(w1.T@xnT+b1),
    c_tile = (hT.T @ w2) * gate, stored bf16 to a per-expert contributions
    DRAM tensor at static offsets (no RMW scatter).
  host: y[token] = sum of its 4 expert contributions (the expert-parallel
    "all-reduce the weighted accumulation" step, done during unshard).
"""

import numpy as np
import ml_dtypes

import concourse.bass as bass
import concourse.mybir as mybir
import concourse.tile as tile
from concourse import bacc
from concourse.bass_utils import run_bass_kernel_spmd
from concourse.masks import make_identity

# Problem shape (fixed by the task).
T, D, H, OUT = 32768, 1024, 2048, 1024
E, K = 8, 4
EPS = 1e-5

NCORES = 8
P = 128
TLOC = T // NCORES          # tokens per core
NTILE = TLOC // P           # 128-token tiles per core (32)
DC = D // P                 # 8 contraction chunks for D
HC = H // P                 # 16 chunks for H
OC = OUT // 512             # 2 output column blocks of 512

BF = mybir.dt.bfloat16
F32 = mybir.dt.float32

_PROGRAM_CACHE = {}

# test-harness hooks (ignored by graders that just call kernel()):
TRACE = False           # set True to request an NTFF trace / HW exec time
LAST_RESULTS = None     # BassKernelResults of the most recent run


BLK = 512
JT = BLK // P            # 128-token sub-tiles per block
PSH_BUFS = 4
OOB = TLOC               # pad index: one past the last valid row (skipped via bounds_check)


def _expert_blocks(ntiles: int, last_bsz: int, tail_small: bool):
    """Split an expert's slot tiles into (offset, jn, bsz) chunks of <=4 tiles,
    distributed evenly so no chunk is tiny. The final chunk's bsz is the exact
    max token count so padded slots aren't streamed through the matmuls. With
    tail_small (last expert), the final chunk is a single tile so the kernel's
    drain tail is short."""
    tail = []
    if tail_small and ntiles > 1:
        tail = [(ntiles - 1, 1, last_bsz - (ntiles - 1) * P)]
        ntiles -= 1
        last_bsz = ntiles * P
    nb = (ntiles + 3) // 4
    base, rem = divmod(ntiles, nb)
    blocks, s0 = [], 0
    for b in range(nb):
        jn = base + (1 if b < rem else 0)
        bsz = jn * P
        if b == nb - 1:
            bsz = last_bsz - s0 * P
            assert (jn - 1) * P < bsz <= jn * P
        blocks.append((s0, jn, bsz))
        s0 += jn
    return blocks + tail


def _make_blocks(tiles_per_expert, last_bsz):
    blocks = []
    for e in range(E):
        for s0, jn, bsz in _expert_blocks(tiles_per_expert[e], last_bsz[e],
                                          e == E - 1):
            blocks.append((e, s0, jn, bsz))
    return blocks


def build_sparse_program(apply_gamma: bool, apply_beta: bool,
                         tiles_per_expert: tuple, last_bsz: tuple,
                         block_lims: tuple):
    """Top-4 sparse FFN. Host supplies per-expert token index lists (padded
    with OOB), slot-ordered fp32 gates, and per expert: the tile count, exact
    max token count, and how many 128-row xn tiles its gathers may touch
    (tokens are ordered by first routed expert). Expert contributions are
    stored (bf16) to per-expert DRAM tensors; host does the final sum."""
    NSLOT = max(tiles_per_expert)
    nc = bacc.Bacc(None, target_bir_lowering=False, debug=False, num_devices=NCORES)

    x = nc.declare_dram_parameter("x", [TLOC, D], BF, isOutput=False)
    w1t = nc.declare_dram_parameter("w1t", [E, P, DC, H], BF, isOutput=False)
    w2t = nc.declare_dram_parameter("w2t", [E, P, HC, OUT], BF, isOutput=False)
    b1t = nc.declare_dram_parameter("b1t", [E, P, HC], F32, isOutput=False)
    gamma = nc.declare_dram_parameter("gamma", [D], F32, isOutput=False)
    beta = nc.declare_dram_parameter("beta", [D], F32, isOutput=False)
    idxt = nc.declare_dram_parameter("idxt", [P, E, NSLOT], mybir.dt.int32,
                                     isOutput=False)
    gslot = nc.declare_dram_parameter("gslot", [P, E, NSLOT], F32,
                                      isOutput=False)

    contribs = [
        nc.declare_dram_parameter(f"contrib{e}", [tiles_per_expert[e] * P, OUT],
                                  BF, isOutput=True)
        for e in range(E)
    ]

    xn_dram = nc.dram_tensor("xn_dram", [TLOC, D], BF)

    blocks = _make_blocks(tiles_per_expert, last_bsz)
    NB = len(blocks)
    assert len(block_lims) == NB

    with tile.TileContext(nc) as tc:
        with (
            tc.tile_pool(name="singles", bufs=1) as singles,
            tc.tile_pool(name="p1", bufs=3) as p1,
            tc.tile_pool(name="p1s", bufs=6) as p1s,
            tc.tile_pool(name="w1pool", bufs=2) as w1pool,
            tc.tile_pool(name="wpool", bufs=2) as wpool,
            tc.tile_pool(name="hpool", bufs=2) as hpool,
            tc.tile_pool(name="ypool", bufs=2) as ypool,
            tc.tile_pool(name="xgtpool", bufs=2) as xgtpool,
            tc.tile_pool(name="xgpool", bufs=6) as xgpool,
            tc.tile_pool(name="ps_tr", bufs=2, space="PSUM") as ps_tr,
            tc.tile_pool(name="ps_h", bufs=PSH_BUFS, space="PSUM") as ps_h,
            tc.tile_pool(name="ps_y", bufs=2, space="PSUM") as ps_y,
        ):
            ident_bf = singles.tile([P, P], BF)
            make_identity(nc, ident_bf)
            eps_t = singles.tile([P, 1], F32)
            nc.vector.memset(eps_t, EPS)
            idx_sb = singles.tile([P, E, NSLOT], mybir.dt.int32)
            nc.gpsimd.dma_start(out=idx_sb, in_=idxt[:, :, :])
            g_sb = singles.tile([P, E, NSLOT], F32)
            nc.gpsimd.dma_start(out=g_sb, in_=gslot[:, :, :])


            if apply_gamma:
                gam_sb = singles.tile([P, D], F32)
                nc.sync.dma_start(
                    out=gam_sb,
                    in_=bass.AP(tensor=gamma.tensor, offset=gamma.offset,
                                ap=[[0, P], *gamma.ap]))
            if apply_beta:
                bet_sb = singles.tile([P, D], F32)
                nc.sync.dma_start(
                    out=bet_sb,
                    in_=bass.AP(tensor=beta.tensor, offset=beta.offset,
                                ap=[[0, P], *beta.ap]))

            # block 0 of expert 0 covers rows 0..511 exactly (tokens are
            # first-expert ordered), so phase-1 tiles 0..3 PE-transpose their
            # xn straight into its xgT: no spill-wait, no gather round-trip.
            b0_direct = (blocks[0] == (0, 0, JT, BLK))
            xgT0 = None
            if b0_direct:
                xgT0 = xgtpool.tile([P, DC, BLK], BF, tag="xgT",
                                    name="xgT0")

            # ---------------- phase 1 tile: LN + spill ------------------------
            def phase1_tile(i):
                xt = p1.tile([P, D], BF, tag="xt")
                nc.sync.dma_start(out=xt, in_=x[i * P:(i + 1) * P, :])

                st = p1s.tile([P, 2, 6], F32, tag="st")
                nc.vector.bn_stats(out=st[:, 0, :], in_=xt[:, 0:512])
                nc.vector.bn_stats(out=st[:, 1, :], in_=xt[:, 512:1024])
                mv = p1s.tile([P, 2], F32, tag="mv")
                nc.vector.bn_aggr(out=mv, in_=st)
                rstd = p1s.tile([P, 1], F32, tag="rstd")
                nc.scalar.activation(out=rstd, in_=mv[:, 1:2],
                                     func=mybir.ActivationFunctionType.Sqrt,
                                     bias=eps_t)
                nc.vector.reciprocal(out=rstd, in_=rstd)

                xnb = p1.tile([P, D], BF, tag="xnb")
                if apply_gamma or apply_beta:
                    nc.vector.tensor_scalar(out=xnb, in0=xt,
                                            scalar1=mv[:, 0:1], scalar2=rstd,
                                            op0=mybir.AluOpType.subtract,
                                            op1=mybir.AluOpType.mult)
                    if apply_gamma:
                        nc.vector.tensor_mul(out=xnb, in0=xnb, in1=gam_sb)
                    if apply_beta:
                        nc.vector.tensor_add(out=xnb, in0=xnb, in1=bet_sb)
                else:
                    # fused LN on the scalar engine: xn = x*rstd + (-mu*rstd)
                    negmr = p1s.tile([P, 1], F32, tag="negmr")
                    nc.vector.tensor_scalar(out=negmr, in0=mv[:, 0:1],
                                            scalar1=rstd, scalar2=-1.0,
                                            op0=mybir.AluOpType.mult,
                                            op1=mybir.AluOpType.mult)
                    for hf in range(2):
                        nc.scalar.activation(
                            out=xnb[:, hf * 512:(hf + 1) * 512],
                            in_=xt[:, hf * 512:(hf + 1) * 512],
                            func=mybir.ActivationFunctionType.Identity,
                            scale=rstd, bias=negmr)
                nc.gpsimd.dma_start(out=xn_dram[i * P:(i + 1) * P, :], in_=xnb)
                if b0_direct and i < JT:
                    for dc in range(DC):
                        pstb = ps_tr.tile([P, P], BF, tag="pstb")
                        nc.tensor.transpose(pstb, xnb[:, dc * P:(dc + 1) * P],
                                            ident_bf)
                        nc.vector.tensor_copy(
                            out=xgT0[:, dc, i * P:(i + 1) * P], in_=pstb)

            emitted = [0]

            def ensure_tiles(n):
                while emitted[0] < min(n, NTILE):
                    phase1_tile(emitted[0])
                    emitted[0] += 1

            # ---------------- phase 2: per-expert sparse FFN -------------------
            # Gathers run two blocks ahead (xg ring throttles); stage A(b) =
            # transpose+w1+gelu; stage B(b) = w2+gate-scale+contrib store;
            # A(b+1) issued before B(b). Next expert's weights are DMA'd on the
            # scalar ring one block before they're needed (bufs sized so the
            # ring slot belongs to the expert before last -> no issue-order
            # hazard). Remaining LN tiles are drip-fed between blocks.

            expert_state = {}

            # weight loads are chunked into ~512KB pieces so their DMA
            # packets round-robin fairly with the small x/xn/gather DMAs
            # (one 4MB dma_start = 32KB descriptors that hog each SDMA
            # engine for ~10us at packet granularity)
            def load_w1(e):
                # chunked along H so stage_a's hc-ordered matmuls can start
                # once the first chunk lands instead of after the full 4MB
                w1sb = w1pool.tile([P, DC, H], BF, tag="w1")
                hh = H // DC
                for k in range(DC):
                    nc.scalar.dma_start(
                        out=w1sb[:, :, k * hh:(k + 1) * hh],
                        in_=w1t[e, :, :, k * hh:(k + 1) * hh])
                b1sb = p1.tile([P, HC], F32, tag="b1")
                nc.scalar.dma_start(out=b1sb, in_=b1t[e, :, :])
                expert_state[e] = {"w1sb": w1sb, "b1sb": b1sb}

            def load_w2(e):
                w2sb = wpool.tile([P, HC, OUT], BF, tag="w2")
                for k in range(DC):
                    nc.scalar.dma_start(out=w2sb[:, 2 * k:2 * k + 2, :],
                                        in_=w2t[e, :, 2 * k:2 * k + 2, :])
                expert_state[e]["w2sb"] = w2sb

            gq = {}
            gathered = [-1]

            def prefetch_gathers(upto):
                while gathered[0] < min(upto, NB - 1):
                    gathered[0] += 1
                    bi = gathered[0]
                    e, s0, jn, bsz = blocks[bi]
                    ensure_tiles(block_lims[bi])
                    if bi == 0 and b0_direct:
                        gq[bi] = None
                        continue
                    lim = block_lims[bi] * P
                    tiles = []
                    for j in range(jn):
                        idx_ap = idx_sb[:, e, s0 + j:s0 + j + 1]
                        xg = xgpool.tile([P, D], BF, tag="xg")
                        nc.gpsimd.indirect_dma_start(
                            out=xg[:, :], out_offset=None,
                            in_=xn_dram[0:lim, :],
                            in_offset=bass.IndirectOffsetOnAxis(ap=idx_ap,
                                                                axis=0),
                            bounds_check=lim - 1, oob_is_err=False)
                        tiles.append(xg)
                    gq[bi] = tiles

            def stage_a(bi):
                e, s0, jn, bsz = blocks[bi]
                st = expert_state[e]
                xgs = gq.pop(bi)
                if bi == 0 and b0_direct:
                    xgT = xgT0
                else:
                    xgT = xgtpool.tile([P, DC, BLK], BF, tag="xgT")
                    for j in range(jn):
                        for dc in range(DC):
                            pstb = ps_tr.tile([P, P], BF, tag="pstb")
                            nc.tensor.transpose(
                                pstb, xgs[j][:, dc * P:(dc + 1) * P], ident_bf)
                            nc.vector.tensor_copy(
                                out=xgT[:, dc, j * P:(j + 1) * P], in_=pstb)

                hT = hpool.tile([P, HC, BLK], BF, tag="hT")
                for hc in range(HC):
                    psh = ps_h.tile([P, BLK], F32, tag="psh")
                    for dc in range(DC):
                        nc.tensor.matmul(
                            psh[:, :bsz],
                            lhsT=st["w1sb"][:, dc, hc * P:(hc + 1) * P],
                            rhs=xgT[:, dc, :bsz],
                            start=(dc == 0), stop=(dc == DC - 1))
                    nc.scalar.activation(out=hT[:, hc, :bsz], in_=psh[:, :bsz],
                                         func=mybir.ActivationFunctionType.Gelu,
                                         bias=st["b1sb"][:, hc:hc + 1])
                return (e, s0, jn, bsz, hT, st)

            def stage_b(pend):
                e, s0, jn, bsz, hT, st = pend
                for j in range(jn):
                    rows = min(P, bsz - j * P)
                    yt = ypool.tile([P, OUT], BF, tag="yt")
                    for oc in range(OC):
                        psy = ps_y.tile([P, 512], F32, tag="psy")
                        for hc in range(HC):
                            nc.tensor.matmul(
                                psy[:rows, :],
                                lhsT=hT[:, hc, j * P:j * P + rows],
                                rhs=st["w2sb"][:, hc, oc * 512:(oc + 1) * 512],
                                start=(hc == 0), stop=(hc == HC - 1))
                        nc.vector.tensor_scalar_mul(
                            out=yt[:rows, oc * 512:(oc + 1) * 512],
                            in0=psy[:rows, :],
                            scalar1=g_sb[:rows, e, s0 + j:s0 + j + 1])
                    r0 = (s0 + j) * P
                    nc.sync.dma_start(out=contribs[e][r0:r0 + rows, :],
                                      in_=yt[:rows, :])

            ensure_tiles(2)
            load_w1(0)
            pending = None
            for bi in range(NB):
                e = blocks[bi][0]
                if bi == 1:
                    load_w2(0)
                if bi + 1 < NB and blocks[bi + 1][0] != e:
                    load_w1(blocks[bi + 1][0])
                    load_w2(blocks[bi + 1][0])
                prefetch_gathers(bi + 2)
                nxt = stage_a(bi)
                if pending is not None:
                    stage_b(pending)
                pending = nxt
                ensure_tiles(emitted[0] + 3)
            stage_b(pending)

    nc.compile()
    return nc


def _assign_cores(sel):
    """Balanced token->core assignment: group tokens by expert signature and
    deal each group round-robin across cores, so per-(core,expert) routed
    counts are all ~N_e/8. Returns [T] core ids (exactly TLOC per core)."""
    sig = sel.astype(np.uint32).dot(1 << np.arange(E, dtype=np.uint32))
    order = np.argsort(sig, kind="stable")
    cores = np.empty(T, np.int64)
    cores[order] = np.arange(T, dtype=np.int64) % NCORES
    return cores


def _plan_routing(x, ln_gamma, ln_beta, router_w, router_b):
    """Host-side routing plan. Returns per-core token id lists (balanced
    assignment, ordered by first routed expert), per-(core,expert) index lists
    (positions within the core, padded with OOB), slot-ordered fp32 gates,
    per-expert tile counts / exact max counts / row limits, and the dense
    [T, E] gate matrix (for the b2 term)."""
    mu = x.mean(axis=1, keepdims=True)
    var = ((x - mu) ** 2).mean(axis=1, keepdims=True)
    xn = (x - mu) / np.sqrt(var + EPS) * ln_gamma + ln_beta
    logits = xn.astype(np.float32) @ router_w + router_b
    order = np.argsort(-logits, axis=1, kind="stable")[:, :K]     # [T, K]
    sel = np.zeros((x.shape[0], E), dtype=bool)
    np.put_along_axis(sel, order, True, axis=1)
    top = np.take_along_axis(logits, order, axis=1)
    gk = np.exp(top - top.max(axis=1, keepdims=True))
    gk = (gk / gk.sum(axis=1, keepdims=True)).astype(np.float32)
    gates = np.zeros((T, E), dtype=np.float32)
    np.put_along_axis(gates, order, gk, axis=1)

    cores = _assign_cores(sel)
    first_e = np.argmax(sel, axis=1)          # min routed expert per token

    tok_ids = []
    for c in range(NCORES):
        ids = np.nonzero(cores == c)[0]
        ids = ids[np.argsort(first_e[ids], kind="stable")]
        tok_ids.append(ids.astype(np.int64))

    counts = np.array([sel[tok_ids[c]].sum(axis=0) for c in range(NCORES)])
    tiles_per_expert = tuple(int(t) for t in
                             (counts.max(axis=0) + P - 1) // P)
    last_bsz = tuple(int(v) for v in counts.max(axis=0))
    nslot = max(tiles_per_expert)

    idxts, gslots, tok_lists, idx_raw = [], [], [], []
    for c in range(NCORES):
        sel_c = sel[tok_ids[c]]
        gates_c = gates[tok_ids[c]]
        idx = np.full((E, nslot * P), OOB, dtype=np.int32)
        gsl = np.zeros((E, nslot * P), dtype=np.float32)
        lists = []
        for e in range(E):
            toks = np.nonzero(sel_c[:, e])[0].astype(np.int32)
            idx[e, :toks.size] = toks
            gsl[e, :toks.size] = gates_c[toks, e]
            lists.append(toks)
        tok_lists.append(lists)
        # [E, nslot*P] -> [P, E, NSLOT] with slot s = (slot_tile, p)
        idxts.append(np.ascontiguousarray(
            idx.reshape(E, nslot, P).transpose(2, 0, 1)))
        gslots.append(np.ascontiguousarray(
            gsl.reshape(E, nslot, P).transpose(2, 0, 1)))
        idx_raw.append(idx)

    # per-block gather row limits: the highest row any core's indices in the
    # block touch (tile-rounded). Tokens are first-expert ordered, so early
    # experts' early blocks only need the first few LN tiles.
    block_lims = []
    for e, s0, jn, bsz in _make_blocks(tiles_per_expert, last_bsz):
        hi = 0
        for c in range(NCORES):
            seg = idx_raw[c][e, s0 * P:(s0 + jn) * P]
            seg = seg[seg < OOB]
            if seg.size:
                hi = max(hi, int(seg.max()) + 1)
        block_lims.append(max(1, -(-hi // P)))
    block_lims = tuple(block_lims)
    return (tok_ids, idxts, gslots, tok_lists, tiles_per_expert, last_bsz,
            block_lims, gates)


def _prep_weights(w1, w2, b1):
    w1t = np.ascontiguousarray(
        w1.reshape(E, DC, P, H).transpose(0, 2, 1, 3)).astype(ml_dtypes.bfloat16)
    w2t = np.ascontiguousarray(
        w2.reshape(E, HC, P, OUT).transpose(0, 2, 1, 3)).astype(ml_dtypes.bfloat16)
    b1t = np.ascontiguousarray(
        b1.reshape(E, HC, P).transpose(0, 2, 1)).astype(np.float32)
    return w1t, w2t, b1t


def kernel(x, ln_gamma, ln_beta, router_w, router_b, w1, b1, w2, b2):
    x = np.asarray(x, dtype=np.float32)
    ln_gamma = np.asarray(ln_gamma, dtype=np.float32)
    ln_beta = np.asarray(ln_beta, dtype=np.float32)
    router_w = np.asarray(router_w, dtype=np.float32)
    router_b = np.asarray(router_b, dtype=np.float32)
    w1 = np.asarray(w1, dtype=np.float32)
    b1 = np.asarray(b1, dtype=np.float32)
    w2 = np.asarray(w2, dtype=np.float32)
    b2 = np.asarray(b2, dtype=np.float32)

    apply_gamma = not np.all(ln_gamma == 1.0)
    apply_beta = not np.all(ln_beta == 0.0)

    (tok_ids, idxts, gslots, tok_lists, tiles_per_expert, last_bsz,
     block_lims, gates) = _plan_routing(x, ln_gamma, ln_beta, router_w,
                                        router_b)

    flags = (apply_gamma, apply_beta)
    key = ("v6", *flags, tiles_per_expert, last_bsz, block_lims)
    if key not in _PROGRAM_CACHE:
        _PROGRAM_CACHE[key] = build_sparse_program(
            *flags, tiles_per_expert, last_bsz, block_lims)
    nc = _PROGRAM_CACHE[key]

    w1t, w2t, b1t = _prep_weights(w1, w2, b1)

    in_maps = []
    for c in range(NCORES):
        m = {
            "x": np.ascontiguousarray(
                x[tok_ids[c]].astype(ml_dtypes.bfloat16)),
            "w1t": w1t, "w2t": w2t, "b1t": b1t,
            "gamma": ln_gamma, "beta": ln_beta,
            "idxt": idxts[c],
            "gslot": gslots[c],
        }
        in_maps.append(m)

    global LAST_RESULTS
    res = run_bass_kernel_spmd(nc, in_maps, list(range(NCORES)), trace=TRACE)
    LAST_RESULTS = res

    # unshard + the expert-parallel weighted-accumulation reduction
    y = np.zeros((T, OUT), dtype=np.float32)
    for c in range(NCORES):
        rc = res.results[c]
        ids = tok_ids[c]
        for e in range(E):
            toks = tok_lists[c][e]
            if toks.size == 0:
                continue
            contrib = np.asarray(rc[f"contrib{e}"][:toks.size], dtype=np.float32)
            y[ids[toks]] += contrib

    if not np.all(b2 == 0.0):
        y = y + gates @ b2
    return y.astype(np.float32)
